# revision 35
# baseline (speedup 1.0000x reference)
"""Trainium2 Bass kernel for nn_PlainAttnLayer (dense transformer attention layer).

Computes, per batch b:
    Q = X @ Wq + bq ; K = X @ Wk + bk ; V = gelu_erf(X @ Wv + bv)
    S = Q K^T / sqrt(F) + mask * (-1e9)
    A = softmax(S, axis=-1)            (output 1)
    C = A @ V                          (output 2)

Sharding: 8 cores = 4 batches x 2 query-row halves. Each core computes the
full K/V for its batch and one half (1024 rows) of the queries. Per-core
inputs are permuted so its query rows always come first in `x`; the attn
output columns are un-permuted on the host.

Host-side data staging (part of kernel()): x is passed pre-transposed
([D, S]) and the weights/biases re-laid-out so every SBUF load is one
contiguous chunk per partition; no on-device transposes are needed for the
projections.

On-chip dataflow (per core):
  - X^T streamed into SBUF in 512-row quarter tiles (triple buffered).
  - Q^T/K^T = matmul(lhsT=W-column-slices, rhs=X^T) as float32r (FP22
    multiply, fp32 accumulate - full PE rate for moving dim >= 256).
  - V (natural layout) = matmul(lhsT=X^T tiles, rhs=Wv slices) as float32r,
    bias added on DVE, gelu on ACT, stored bf16.
  - scores = matmul(lhsT=Q^T, rhs=K^T) f32r; mask folded in with one DVE
    scalar_tensor_tensor reading PSUM; softmax uses ACT Exp with fused
    row-sum accumulation; attention written fp32.
  - A^T via PE transposes (f32r); C = matmul(lhsT=A^T, rhs=V) in bf16 with
    the 1/rowsum normalization folded into the PSUM eviction.
"""

import math

import numpy as np

try:
    import concourse.bass as bass  # noqa: F401
except ImportError:  # pragma: no cover
    import sys

    sys.path.insert(0, "/opt/trn_rl_repo")
    import concourse.bass as bass  # noqa: F401

import concourse.tile as tile
from concourse import bacc, bass_utils, mybir
from concourse.masks import make_identity

F32 = mybir.dt.float32
F32R = mybir.dt.float32r
BF16 = mybir.dt.bfloat16
I32 = mybir.dt.int32
AF = mybir.ActivationFunctionType
ALU = mybir.AluOpType
AX = mybir.AxisListType
P = 128

# Problem dimensions (hardcoded; see spec)
B, S, D, F, H = 4, 2048, 1024, 1024, 1024
SQ = S // 2  # query rows per core
N_CORES = 8


def _build_tile(tc, aps, S, D, F, H, SQ):
    """Emit the per-core program. aps: dict of DRAM APs."""
    nc = tc.nc
    x, mask = aps["x"], aps["mask"]
    wq, wk, wv = aps["wq"], aps["wk"], aps["wv"]
    bq, bk, bv2d = aps["bq"], aps["bk"], aps["bv2d"]
    attn_out, ctx_out = aps["attn_out"], aps["ctx_out"]

    DT, FT, ST, IT = D // P, F // P, S // P, SQ // P
    NH = 2  # W-load passes (halves); X^T streamed in quarter tiles
    SH = S // NH
    QSZ = min(512, SH)  # quarter tile width (kv rows)
    QH = SH // QSZ  # quarters per half
    assert SQ % SH == 0, "query rows must cover whole halves"
    CH = min(512, S, SQ, H)  # moving-dim chunk (phase 2)
    CHK = QSZ  # K/Q moving chunk within a quarter
    CHV = min(256, CH)  # smaller chunk for V projection (SBUF pressure)
    scale = 1.0 / math.sqrt(F)
    maskc = -1e9 / scale  # fold mask*(-1e9) pre-scaling

    import contextlib

    with contextlib.ExitStack() as ctx:
        const = ctx.enter_context(tc.tile_pool(name="const", bufs=1))
        identity = const.tile([P, P], F32)
        make_identity(nc, identity)
        identity_r = const.tile([P, P], F32R)
        nc.vector.tensor_copy(identity_r, identity)
        bq_sb = const.tile([P, FT], F32)
        bk_sb = const.tile([P, FT], F32)
        bv_sb = const.tile([P, H], F32)

        persist = ctx.enter_context(tc.tile_pool(name="persist", bufs=1))
        qt = persist.tile([P, FT, SQ], F32R)  # Q^T: [f-part, ftile, qrow]
        kt = persist.tile([P, FT, S], F32R)  # K^T: [f-part, ftile, kvrow]
        v = persist.tile([P, ST, H], BF16)  # V:   [j-part, jtile, h]

        # ---------- Phase 1: X^T quarter loads + Q^T/K^T/V projections ----
        with tc.tile_pool(name="wcol", bufs=2) as wcol_pool, \
                tc.tile_pool(name="wvs", bufs=2) as wv_pool, \
                tc.tile_pool(name="vtmp", bufs=2) as vtmp_pool, \
                tc.tile_pool(name="xt", bufs=3) as xt_pool, \
                tc.tile_pool(name="pps", bufs=4, space="PSUM") as pps_pool:

            def load_quarter(q):
                t = xt_pool.tile([P, DT, QSZ], F32R, tag="xt", name=f"xt_{q}")
                for dt in range(DT):
                    nc.sync.dma_start(
                        t[:, dt, :], x[dt * P:(dt + 1) * P,
                                       q * QSZ:(q + 1) * QSZ])
                return t

            first = True
            wk_col0 = None
            for h in range(NH):
                if first:
                    # first weight tile ahead of everything: the opening
                    # matmul group needs it plus quarter 0
                    wk_col0 = wcol_pool.tile([P, DT, P], F32R, tag="wcol",
                                             name="wkc_0_0")
                    nc.sync.dma_start(wk_col0, wk[:, 0])
                qs = [load_quarter(h * QH + qq) for qq in range(QH)]
                if first:
                    # small constant loads queue behind the first quarter
                    nc.sync.dma_start(bq_sb, bq)
                    nc.sync.dma_start(bk_sb, bk)
                    nc.sync.dma_start(bv_sb, bv2d)
                    first = False

                # K^T for this half's rows
                for ft in range(FT):
                    if h == 0 and ft == 0:
                        wk_col = wk_col0
                    else:
                        wk_col = wcol_pool.tile([P, DT, P], F32R, tag="wcol",
                                                name=f"wkc_{h}_{ft}")
                        nc.sync.dma_start(wk_col, wk[:, ft])
                    for jc in range(QH):
                        ps = pps_pool.tile([P, CHK], F32, tag="pps",
                                           name=f"kps_{h}_{ft}_{jc}")
                        for dt in range(DT):
                            nc.tensor.matmul(
                                ps, wk_col[:, dt], qs[jc][:, dt, :],
                                start=(dt == 0), stop=(dt == DT - 1))
                        c0 = h * SH + jc * CHK
                        nc.scalar.activation(kt[:, ft, c0:c0 + CHK], ps,
                                             AF.Identity,
                                             bias=bk_sb[:, ft:ft + 1])

                # Q^T for this half's query rows (if any)
                if h * SH < SQ:
                    for ft in range(FT):
                        wq_col = wcol_pool.tile([P, DT, P], F32R, tag="wcol",
                                                name=f"wqc_{h}_{ft}")
                        nc.sync.dma_start(wq_col, wq[:, ft])
                        for jc in range(QH):
                            c0 = h * SH + jc * CHK
                            ps = pps_pool.tile([P, CHK], F32, tag="pps",
                                               name=f"qps_{h}_{ft}_{jc}")
                            for dt in range(DT):
                                nc.tensor.matmul(
                                    ps, wq_col[:, dt], qs[jc][:, dt, :],
                                    start=(dt == 0), stop=(dt == DT - 1))
                            nc.scalar.activation(
                                qt[:, ft, c0:c0 + CHK], ps,
                                AF.Identity, bias=bq_sb[:, ft:ft + 1])

                # V for this half's rows (natural layout, bf16)
                for hc in range(H // CHV):
                    wv_sb = wv_pool.tile([P, DT, CHV], F32R, tag="wvs",
                                         name=f"wvs_{h}_{hc}")
                    nc.sync.dma_start(wv_sb, wv[:, hc])
                    for jl in range(SH // P):
                        jt = h * (SH // P) + jl
                        xq = qs[(jl * P) // QSZ]
                        joff = (jl * P) % QSZ
                        ps = pps_pool.tile([P, CHV], F32, tag="pps",
                                           name=f"vps_{h}_{hc}_{jl}")
                        for dt in range(DT):
                            nc.tensor.matmul(
                                ps, xq[:, dt, joff:joff + P], wv_sb[:, dt],
                                start=(dt == 0), stop=(dt == DT - 1))
                        vt = vtmp_pool.tile([P, CHV], F32, tag="vtmp",
                                            name=f"vtmp_{h}_{hc}_{jl}")
                        nc.vector.tensor_add(vt, ps,
                                             bv_sb[:, hc * CHV:(hc + 1) * CHV])
                        nc.scalar.activation(v[:, jt, hc * CHV:(hc + 1) * CHV],
                                             vt, AF.Gelu)

        # ---------- Phase 2: scores, softmax, A^T, context ----------
        with tc.tile_pool(name="mk", bufs=2) as mask_pool, \
                tc.tile_pool(name="row", bufs=5) as row_pool, \
                tc.tile_pool(name="stat", bufs=12) as stat_pool, \
                tc.tile_pool(name="at", bufs=2) as at_pool, \
                tc.tile_pool(name="csb", bufs=3) as c_pool, \
                tc.tile_pool(name="sps", bufs=4, space="PSUM") as sps_pool, \
                tc.tile_pool(name="atps", bufs=2, space="PSUM") as atps_pool, \
                tc.tile_pool(name="cps", bufs=2, space="PSUM") as cps_pool:
            for it in range(IT):
                mk = mask_pool.tile([P, S], I32, tag="mk", name=f"mk_{it}")
                nc.sync.dma_start(mk, mask[it * P:(it + 1) * P, :])

                # scores (pre-scale) with mask folded in: T = S_raw + maskc*m
                tbuf = row_pool.tile([P, S], F32, tag="row", name=f"t_{it}")
                for jc in range(S // CH):
                    ps = sps_pool.tile([P, CH], F32, tag="sps",
                                       name=f"sps_{it}_{jc}")
                    for ft in range(FT):
                        nc.tensor.matmul(
                            ps, qt[:, ft, it * P:(it + 1) * P],
                            kt[:, ft, jc * CH:(jc + 1) * CH],
                            start=(ft == 0), stop=(ft == FT - 1))
                    nc.vector.scalar_tensor_tensor(
                        tbuf[:, jc * CH:(jc + 1) * CH],
                        mk[:, jc * CH:(jc + 1) * CH], float(maskc), ps,
                        op0=ALU.mult, op1=ALU.add)

                # softmax over the full row (free dim)
                negmax = stat_pool.tile([P, 1], F32, tag="stat",
                                        name=f"negmax_{it}")
                nc.vector.reduce_max(negmax, tbuf, axis=AX.X, negate=True)
                negmax_s = stat_pool.tile([P, 1], F32, tag="stat",
                                          name=f"negmaxs_{it}")
                nc.vector.tensor_scalar_mul(negmax_s, negmax, float(scale))
                aexp = row_pool.tile([P, S], F32R, tag="row", name=f"aexp_{it}")
                rowsum = stat_pool.tile([P, 1], F32, tag="stat",
                                        name=f"rowsum_{it}")
                nc.scalar.activation(aexp, tbuf, AF.Exp, bias=negmax_s,
                                     scale=float(scale), accum_out=rowsum)
                recip = stat_pool.tile([P, 1], F32, tag="stat",
                                       name=f"recip_{it}")
                nc.vector.reciprocal(recip, rowsum)

                # normalized attention out (fp32)
                anorm = row_pool.tile([P, S], F32, tag="row",
                                      name=f"anorm_{it}")
                nc.vector.tensor_scalar_mul(anorm, aexp, recip)
                nc.sync.dma_start(attn_out[it * P:(it + 1) * P, :], anorm)

                # A^T (unnormalized, bf16) via PE transposes
                at = at_pool.tile([P, ST, P], BF16, tag="at", name=f"at_{it}")
                tgrp = min(4, ST)
                for jt0 in range(0, ST, tgrp):
                    g = min(tgrp, ST - jt0)
                    ps = atps_pool.tile([P, tgrp * P], F32R, tag="atps",
                                        name=f"atps_{it}_{jt0}")
                    for k in range(g):
                        nc.tensor.transpose(
                            ps[:, k * P:(k + 1) * P],
                            aexp[:, (jt0 + k) * P:(jt0 + k + 1) * P], identity_r)
                    nc.vector.tensor_copy(at[:, jt0:jt0 + g, :], ps[:, :g * P])

                # context: C = (A_unnorm @ V) * recip
                for hc in range(H // CH):
                    cp = cps_pool.tile([P, CH], F32, tag="cps",
                                       name=f"cps_{it}_{hc}")
                    for jt in range(ST):
                        nc.tensor.matmul(cp, at[:, jt, :],
                                         v[:, jt, hc * CH:(hc + 1) * CH],
                                         start=(jt == 0), stop=(jt == ST - 1))
                    c_sb = c_pool.tile([P, CH], F32, tag="csb",
                                       name=f"csb_{it}_{hc}")
                    nc.scalar.activation(c_sb, cp, AF.Identity, scale=recip)
                    nc.sync.dma_start(
                        ctx_out[it * P:(it + 1) * P, hc * CH:(hc + 1) * CH],
                        c_sb)


def build_program(S=S, D=D, F=F, H=H, SQ=SQ):
    nc = bacc.Bacc("TRN2", target_bir_lowering=False, debug=False)
    aps = {
        "x": nc.dram_tensor("x", [D, S], F32R, kind="ExternalInput").ap(),
        "mask": nc.dram_tensor("mask", [SQ, S], I32, kind="ExternalInput").ap(),
        "wq": nc.dram_tensor("wq", [P, F // P, D // P, P], F32R,
                              kind="ExternalInput").ap(),
        "wk": nc.dram_tensor("wk", [P, F // P, D // P, P], F32R,
                              kind="ExternalInput").ap(),
        "wv": nc.dram_tensor("wv", [P, H // 256 if H >= 256 else 1, D // P, min(256, H)], F32R,
                              kind="ExternalInput").ap(),
        "bq": nc.dram_tensor("bq", [P, F // P], F32, kind="ExternalInput").ap(),
        "bk": nc.dram_tensor("bk", [P, F // P], F32, kind="ExternalInput").ap(),
        "bv2d": nc.dram_tensor("bv2d", [P, H], F32, kind="ExternalInput").ap(),
        "attn_out": nc.dram_tensor("attn_out", [SQ, S], F32,
                                   kind="ExternalOutput").ap(),
        "ctx_out": nc.dram_tensor("ctx_out", [SQ, H], F32,
                                  kind="ExternalOutput").ap(),
    }
    with tile.TileContext(nc) as tc:
        _build_tile(tc, aps, S, D, F, H, SQ)
    nc.compile()
    return nc


_compiled_nc = None


def _get_nc():
    global _compiled_nc
    if _compiled_nc is None:
        _compiled_nc = build_program()
    return _compiled_nc


def _make_in_maps(inputs, attn_mask, Wq, bq, Wk, bk, Wv, bv):
    inputs = np.ascontiguousarray(np.asarray(inputs, dtype=np.float32))
    attn_mask = np.ascontiguousarray(np.asarray(attn_mask, dtype=np.int32))
    DT, FT = D // P, F // P
    CHV = min(256, H)
    # weight layouts so each per-ftile/hc load is one contiguous chunk per
    # SBUF partition: w_re[p, ft, dt, fi] = W[dt*P + p, ft*P + fi]
    Wq = np.ascontiguousarray(np.asarray(Wq, dtype=np.float32)
                              .reshape(DT, P, FT, P).transpose(1, 2, 0, 3))
    Wk = np.ascontiguousarray(np.asarray(Wk, dtype=np.float32)
                              .reshape(DT, P, FT, P).transpose(1, 2, 0, 3))
    Wv = np.ascontiguousarray(np.asarray(Wv, dtype=np.float32)
                              .reshape(DT, P, H // CHV, CHV)
                              .transpose(1, 2, 0, 3))
    bq = np.ascontiguousarray(
        np.asarray(bq, dtype=np.float32).reshape(FT, P).T)
    bk = np.ascontiguousarray(
        np.asarray(bk, dtype=np.float32).reshape(FT, P).T)
    bv2d = np.ascontiguousarray(
        np.broadcast_to(np.asarray(bv, dtype=np.float32), (P, H)))
    in_maps = []
    for c in range(N_CORES):
        b, half = c // 2, c % 2
        qoff = half * SQ
        xb = inputs[b]
        # permute rows so this core's query rows come first
        xp = np.concatenate([xb[qoff:qoff + SQ], xb[SQ - qoff:2 * SQ - qoff]], 0)
        m = attn_mask[b, qoff:qoff + SQ]
        mp = np.concatenate(
            [m[:, qoff:qoff + SQ], m[:, SQ - qoff:2 * SQ - qoff]], 1)
        in_maps.append({
            "x": np.ascontiguousarray(xp.T),
            "mask": np.ascontiguousarray(mp),
            "wq": Wq, "wk": Wk, "wv": Wv,
            "bq": bq, "bk": bk, "bv2d": bv2d,
        })
    return in_maps


def kernel(inputs, attn_mask, Wq, bq, Wk, bk, Wv, bv):
    nc = _get_nc()
    in_maps = _make_in_maps(inputs, attn_mask, Wq, bq, Wk, bk, Wv, bv)
    res = bass_utils.run_bass_kernel_spmd(nc, in_maps,
                                          core_ids=list(range(N_CORES)))
    context = np.empty((B, S, H), np.float32)
    attn = np.empty((B, S, S), np.float32)
    for c in range(N_CORES):
        b, half = c // 2, c % 2
        qoff = half * SQ
        r = res.results[c]
        context[b, qoff:qoff + SQ] = r["ctx_out"]
        a = r["attn_out"]
        attn[b, qoff:qoff + SQ, qoff:qoff + SQ] = a[:, :SQ]
        attn[b, qoff:qoff + SQ, SQ - qoff:2 * SQ - qoff] = a[:, SQ:]
    return context, attn
